# revision 80
# baseline (speedup 1.0000x reference)
"""Multi-head causal+padded attention on 8 Trainium2 NeuronCores.

Sharding: core c handles batch b = c//2 and head-group g = c%2 (8 of 16 heads).

Key optimization: the pad mask zeroes ~half the sequence positions, and padded
q-rows output exactly 0 while padded k-positions contribute nothing. The host
compacts each batch to its valid positions (order-preserving, so causality in
compressed coordinates is just j <= i), pads to a multiple of 128 (S_c), and
scatters the device results back. All device work (projections, scores, exp,
AV) shrinks from 2048 to ~1152 positions.

The q side is additionally trimmed to the exact max valid count (S_q); the
k/v side keeps the 128-padded S_c, with dead-k rows zeroed so they drop out
of both the attention numerator and denominator.

Device algorithm (per core), all bf16 operands with f32 PSUM accumulation:
  xT [1024, S_c] resident in SBUF; qT/kT = W^T-slices @ xT ([outdim, seq]
  layout, bias-add + bf16 cast on the scalar engine); v in natural
  [seq, outdim] layout, bias-added, pad-masked, stored bf16 augmented with a
  65th column = pad mask, so att^T @ [v|pad] accumulates out^T[d,q] plus the
  softmax denominator (row 64) in one PSUM chain.
  Per q-chunk (<=512 wide) and head pair: score blocks sT[k,q] for both heads
  land in adjacent PSUM banks of one [128,1024] tile and one Exp activation
  (scale=1/8 folded in) covers the pair; the causal mask is applied by
  gpsimd affine_select on diagonal blocks. Pair hp's scores need only
  out-block hp of the q/k projections, so per-ob projection chains interleave
  with the scores passes, and each pair's AV/normalize runs one stage behind
  the next pair's scores/exp — across chunk boundaries — keeping the PE
  dense while the scalar engine streams exps. The <=32-wide tail chunk packs
  all score blocks of a pair into one PSUM bank (single exp) and all 8 AV
  accumulators into another (single recip/broadcast/mul + one scattered DMA).
  k is only projected up to S_q (k > max valid q is never attended; the dead
  kT/x tails are memset instead of computed/shipped). DMAs are batched to ~10
  transfers (HWDGE dispatch costs ~650ns each), wq/x/wk halves interleaved in
  demand order with consts deferred past the critical prefix, so the first
  projection unblocks early; outputs leave as bf16.
"""
import math
import os
import sys

sys.path.insert(0, "/opt/trn_rl_repo")

import numpy as np

E = 1024
D = 64
H = 16          # total heads
HPC = 8         # heads per core
OC = HPC * D    # 512 output dims per core
EB = E // 128   # 8 contraction blocks
B = 4
NCORES = 8
S_FULL = 2048

_cache = {}


def _chunks(s):
    out = []
    q0 = 0
    while q0 < s:
        w = min(512, s - q0)
        out.append((q0, w))
        q0 += w
    return out


def _build_nc(SQ, S):
    from concourse import bacc
    import concourse.tile as tile
    import concourse.mybir as mybir

    F32 = mybir.dt.float32
    BF16 = mybir.dt.bfloat16
    AF = mybir.ActivationFunctionType

    NSB = S // 128  # seq blocks (k/v side, padded)
    chunks = _chunks(SQ)  # q side, trimmed to the real max valid count
    FLAGS = set(os.environ.get("MHA_FLAGS", "").split(","))

    nc = bacc.Bacc("TRN2", target_bir_lowering=False, debug=False,
                   num_devices=NCORES)
    xT = nc.dram_tensor("xT", [E, SQ], BF16, kind="ExternalInput").ap()
    wqT = nc.dram_tensor("wqT", [E, OC], BF16, kind="ExternalInput").ap()
    wkT = nc.dram_tensor("wkT", [E, OC], BF16, kind="ExternalInput").ap()
    wvT = nc.dram_tensor("wvT", [E, OC], BF16, kind="ExternalInput").ap()
    # consts packs [pad (NSB) | bq (4) | bk (4)] column-blocks, p-major
    consts = nc.dram_tensor("consts", [128, S // 128 + 8], F32,
                            kind="ExternalInput").ap()
    bv = nc.dram_tensor("bv", [1, OC], F32, kind="ExternalInput").ap()
    outT = nc.dram_tensor("outT", [OC, S], BF16,
                          kind="ExternalOutput").ap()

    with tile.TileContext(nc) as tc:
        with tc.tile_pool(name="const", bufs=1) as cpool, \
             tc.tile_pool(name="big", bufs=1) as bigpool:

            # ---------------- constants ----------------
            # (their DMAs are issued after the critical wq/x/wk prefix —
            # biases/pad aren't read until the first projection completes)
            consts_sb = cpool.tile([128, NSB + 8], F32, tag="consts")
            pad_col = lambda sb: consts_sb[:, sb:sb + 1]
            bq_col = lambda ob: consts_sb[:, NSB + ob:NSB + ob + 1]
            bk_col = lambda ob: consts_sb[:, NSB + 4 + ob:NSB + 5 + ob]
            bv_row = cpool.tile([1, OC], F32, tag="bv_row")
            bv_tile = cpool.tile([128, OC], F32, tag="bv_tile")

            x_sb = bigpool.tile([128, EB * S], BF16, tag="x_sb")
            qT_sb = bigpool.tile([128, 4 * S], BF16, tag="qT")
            kT_sb = bigpool.tile([128, 4 * S], BF16, tag="kT")
            v_aug = bigpool.tile([128, NSB * HPC * 65], BF16, tag="v_aug")
            v_r = v_aug[:].rearrange("p (b h c) -> p b h c", b=NSB, h=HPC)

            w_sbs = {}
            with tc.tile_pool(name="wpool", bufs=1) as wpool:
                for nm in ("q", "k", "v"):
                    w_sbs[nm] = wpool.tile([128, EB * OC], BF16,
                                           tag=f"w{nm}", name=f"w_{nm}")
                # One DMA per tensor: HWDGE dispatch (~650ns each) dominates
                # small transfers, so batch maximally. wq + x first — the
                # q-projection unblocks on those; wk/wv follow.
                def wview(t):
                    return t[:].rearrange("p (e o) -> p e o", e=EB)
                x_v = x_sb[:].rearrange("p (e s) -> p e s", e=EB)
                x_d = xT.rearrange("(e p) s -> p e s", p=128)
                wq_v = wview(w_sbs["q"])
                wq_d = wqT.rearrange("(e p) o -> p e o", p=128)
                wk_v = wview(w_sbs["k"])
                wk_d = wkT.rearrange("(e p) o -> p e o", p=128)
                # Full-tensor DMAs in demand order. PE's first chain needs
                # all of wq+x anyway; starting later but fully dense beats an
                # early trickle (every PE idle gap resets the clock ramp).
                nc.sync.dma_start(wq_v[:], wq_d)
                nc.sync.dma_start(x_v[:], x_d)
                nc.sync.dma_start(wk_v[:], wk_d)
                nc.sync.dma_start(wview(w_sbs["v"]),
                                  wvT.rearrange("(e p) o -> p e o", p=128))

                # init the 65th (pad) columns of v_aug once
                nc.gpsimd.memset(v_r[:, :, :, 64], 1.0)
                if SQ < S:
                    # dead seq tail: x must be zero (feeds v of dead rows),
                    # kT merely finite (dead-k contributions are pad-zeroed)
                    nc.gpsimd.memset(x_v[:, :, SQ:S], 0.0)
                    nc.gpsimd.memset(
                        kT_sb[:].rearrange("p (o s) -> p o s", o=4)
                        [:, :, SQ:S], 0.0)

                with tc.tile_pool(name="psP", bufs=2, space="PSUM") as psP, \
                     tc.tile_pool(name="attp",
                                  bufs=5 if NSB <= 10 else 2) as attp, \
                     tc.tile_pool(name="work", bufs=4) as work, \
                     tc.tile_pool(name="outp", bufs=3) as outp, \
                     tc.tile_pool(name="psS", bufs=2, space="PSUM") as psS, \
                     tc.tile_pool(name="psAv", bufs=2, space="PSUM") as psAv:

                    x_r = x_sb[:].rearrange("p (e s) -> p e s", e=EB)
                    w_rs = {nm: w_sbs[nm][:].rearrange("p (e o) -> p e o",
                                                       e=EB)
                            for nm in ("q", "k", "v")}

                    def proj_qk(nm, bias_col, dst, ob, c0, c1):
                        # projection of out-block ob, columns [c0, c1), into
                        # [o, s] layout. Bias-add+cast runs on ACT (Identity),
                        # which then flows straight into that pair's exps.
                        if c1 <= c0:
                            return
                        w_r = w_rs[nm]
                        ps = psP.tile([128, 512], F32, tag="ps_proj",
                                      name=f"ps_{nm}{ob}")
                        for eb in range(EB):
                            nc.tensor.matmul(
                                ps[:, 0:c1 - c0],
                                w_r[:, eb, ob * 128:(ob + 1) * 128],
                                x_r[:, eb, c0:c1],
                                start=(eb == 0), stop=(eb == EB - 1))
                        if "dve_bias" in FLAGS:
                            nc.vector.tensor_scalar_add(
                                dst[:, ob * S + c0:ob * S + c1],
                                ps[:, 0:c1 - c0], bias_col(ob))
                        else:
                            nc.scalar.activation(
                                dst[:, ob * S + c0:ob * S + c1],
                                ps[:, 0:c1 - c0], AF.Identity,
                                bias=bias_col(ob))

                    def proj_v(sb):
                        # v for seq block sb, natural [s, o] layout, bf16
                        w_r = w_rs["v"]
                        ps = psP.tile([128, 512], F32, tag="ps_proj")
                        for eb in range(EB):
                            nc.tensor.matmul(
                                ps[:],
                                x_r[:, eb, sb * 128:(sb + 1) * 128],
                                w_r[:, eb, :],
                                start=(eb == 0), stop=(eb == EB - 1))
                        nc.vector.tensor_add(
                            v_r[:, sb, :, 0:64],
                            ps[:].rearrange("p (h c) -> p h c", h=HPC),
                            bv_tile[:].rearrange("p (h c) -> p h c", h=HPC))
                        nc.vector.tensor_scalar_mul(
                            v_aug[:, sb * HPC * 65:(sb + 1) * HPC * 65],
                            v_aug[:, sb * HPC * 65:(sb + 1) * HPC * 65],
                            pad_col(sb))

                    def is_narrow(q0, qw):
                        # slots padded to 32 cols (64B) for engine alignment
                        return (qw <= 32
                                and ((q0 + qw + 127) // 128) * 32 <= 512
                                and "no_narrow" not in FLAGS)

                    def att_slot(q0, qw, kb, i):
                        # column offset of (kb, head-in-pair) in the att tile
                        if is_narrow(q0, qw):
                            return i * 512 + kb * 32
                        return kb * 1024 + i * 512

                    def pass1(q0, qw, hp, att_c):
                        # scores + exp (+causal mask) for one head pair
                        nkb = (q0 + qw + 127) // 128
                        heads = (2 * hp, 2 * hp + 1)
                        narrow = is_narrow(q0, qw)
                        if narrow:
                            # all score blocks of the pair packed in one PSUM
                            # bank -> a single exp for the whole chunk
                            spn = psS.tile([128, 1024], F32, tag="ps_s",
                                           name="spn")
                        for kb in range(nkb):
                            lstart = max(0, kb * 128 - q0)
                            w = qw - lstart
                            if not narrow:
                                sp = psS.tile([128, 1024], F32, tag="ps_s")
                            for i, h in enumerate(heads):
                                ob = h // 2
                                po = (h % 2) * 64
                                dst = (spn[:, i * 512 + kb * 32:
                                           i * 512 + kb * 32 + qw] if narrow
                                       else sp[:, i * 512:i * 512 + w])
                                nc.tensor.matmul(
                                    dst,
                                    kT_sb[po:po + 64,
                                          ob * S + kb * 128:
                                          ob * S + (kb + 1) * 128],
                                    qT_sb[po:po + 64,
                                          ob * S + q0 + lstart:
                                          ob * S + q0 + qw],
                                    start=True, stop=True)
                            if not narrow:
                                sp_r = sp[:].rearrange("p (i w) -> p i w", i=2)
                                at_r = att_c[:, kb * 1024:(kb + 1) * 1024] \
                                    .rearrange("p (i w) -> p i w", i=2)
                                nc.scalar.activation(
                                    at_r[:, :, 0:w], sp_r[:, :, 0:w],
                                    AF.Exp, scale=0.125)
                        if narrow:
                            spn_r = spn[:].rearrange(
                                "p (i k c) -> p i k c", i=2, k=16)
                            atn_r = att_c[:, 0:1024].rearrange(
                                "p (i k c) -> p i k c", i=2, k=16)
                            nc.scalar.activation(
                                atn_r[:, :, 0:nkb, 0:qw],
                                spn_r[:, :, 0:nkb, 0:qw],
                                AF.Exp, scale=0.125)
                        for kb in range(nkb):
                            if kb * 128 < q0:
                                continue
                            # zero att where k_local > q_local on the diagonal
                            # block (gpsimd, off DVE's back)
                            tw = min(128, qw - (kb * 128 - q0))
                            for i in range(2):
                                off = att_slot(q0, qw, kb, i)
                                sl = att_c[:, off:off + tw]
                                nc.gpsimd.affine_select(
                                    out=sl, in_=sl,
                                    compare_op=mybir.AluOpType.is_ge,
                                    fill=0.0, base=0,
                                    pattern=[[1, tw]],
                                    channel_multiplier=-1)

                    def pass2(q0, qw, hp, att_c, o_all, av_all):
                        # att^T @ [v|pad] chain per head, then normalize into
                        # the chunk's batched output tile. Narrow chunks
                        # accumulate all 8 heads into one shared PSUM bank
                        # (64-col slots) and normalize once at the end.
                        nkb = (q0 + qw + 127) // 128
                        heads = (2 * hp, 2 * hp + 1)
                        for i, h in enumerate(heads):
                            if av_all is not None:
                                av = av_all
                                col0 = h * 64
                            else:
                                av = psAv.tile([65, 512], F32, tag="ps_av")
                                col0 = 0
                            for kb in range(nkb):
                                lstart = max(0, kb * 128 - q0)
                                w = qw - lstart
                                off = att_slot(q0, qw, kb, i)
                                nc.tensor.matmul(
                                    av[:, col0 + lstart:col0 + qw],
                                    v_r[:, kb, h, :],
                                    att_c[:, off:off + w],
                                    start=(kb == 0), stop=(kb == nkb - 1))
                            if av_all is not None:
                                continue
                            r1 = work.tile([1, 512], F32, tag="rt", name="r1")
                            nc.vector.reciprocal(r1[:, 0:qw], av[64:65, 0:qw])
                            bc = work.tile([64, 512], F32, tag="bc", name="bc")
                            nc.gpsimd.partition_broadcast(bc[:, 0:qw],
                                                          r1[:, 0:qw])
                            po = (h % 2) * 64
                            nc.vector.tensor_mul(
                                o_all[po:po + 64,
                                      (h // 2) * 512:(h // 2) * 512 + qw],
                                av[0:64, 0:qw], bc[:, 0:qw])

                    # Interleave per-ob projections with that pair's scores
                    # pass — pair hp's scores need only out-block hp of q/k —
                    # then pipeline AV one stage behind the exps, across chunk
                    # boundaries. v-projections slot in before the first AV
                    # needs them.
                    stages = []      # emitted pass1 stages awaiting pass2
                    rem = {}         # chunk -> pass2 stages left before DMA

                    chunk_av = {}

                    def emit_pass2(st):
                        sq0, sqw, shp, satt, so_all, snarrow, sci = st
                        if snarrow and sci not in chunk_av:
                            chunk_av[sci] = psAv.tile([65, 512], F32,
                                                      tag="ps_av",
                                                      name=f"av_all{sci}")
                        sav = chunk_av[sci] if snarrow else None
                        pass2(sq0, sqw, shp, satt, so_all, sav)
                        rem[sci] -= 1
                        if rem[sci] != 0:
                            return
                        if sav is not None:
                            # narrow chunk: one recip/broadcast/mul for all 8
                            # heads over just the live slot columns, then one
                            # scattered DMA
                            def hsl(ap):
                                return ap.rearrange("p (hh c) -> p hh c",
                                                    hh=8)[:, :, 0:sqw]
                            r1 = work.tile([1, 512], F32, tag="rt", name="r1")
                            nc.vector.reciprocal(hsl(r1[:]), hsl(sav[64:65, :]))
                            bc = work.tile([64, 512], F32, tag="bc", name="bc")
                            nc.gpsimd.partition_broadcast(hsl(bc[:]),
                                                          hsl(r1[:]))
                            stg = outp.tile([64, 512], BF16, tag="stg",
                                            name="stg")
                            nc.vector.tensor_mul(hsl(stg[:]),
                                                 hsl(sav[0:64, :]),
                                                 hsl(bc[:]))
                            nc.sync.dma_start(
                                outT.rearrange("(hh d) s -> d hh s", d=64)
                                    [:, :, sq0:sq0 + sqw],
                                stg[:].rearrange("p (hh c) -> p hh c", hh=8)
                                    [:, :, 0:sqw])
                        else:
                            nc.sync.dma_start(
                                outT.rearrange("(a p) s -> p a s", p=128)
                                    [:, :, sq0:sq0 + sqw],
                                so_all[:].rearrange("p (a s) -> p a s",
                                                    a=4)[:, :, 0:sqw])

                    kdone = 0
                    for ci, (q0, qw) in enumerate(chunks):
                        kend = min(((q0 + qw + 127) // 128) * 128, S)
                        nkb = (q0 + qw + 127) // 128
                        narrow = is_narrow(q0, qw)
                        o_all = None if narrow else outp.tile(
                            [128, 4 * 512], BF16, tag="osb", name=f"o_all{ci}")
                        rem[ci] = 4
                        last = ci == len(chunks) - 1 and narrow
                        for hp in range(4):
                            proj_qk("q", bq_col, qT_sb, hp, q0, q0 + qw)
                            proj_qk("k", bk_col, kT_sb, hp,
                                    kdone, min(kend, SQ))
                            # narrow chunks use only the first 1024 att cols
                            att_c = attp.tile(
                                [128, 1024 if narrow else nkb * 1024], BF16,
                                tag="attn" if narrow else "att",
                                name=f"att{ci}_{hp}")
                            pass1(q0, qw, hp, att_c)
                            if hp == 1 and kend > kdone:
                                for sb in range(kdone // 128, kend // 128):
                                    proj_v(sb)
                            stages.append((q0, qw, hp, att_c, o_all, narrow,
                                           ci))
                            # on the last chunk, emit every pass1 before
                            # draining: the trailing AV chains then run as one
                            # dense block against already-finished exps
                            if not last and len(stages) > 2:
                                emit_pass2(stages.pop(0))
                        kdone = kend
                    while stages:
                        emit_pass2(stages.pop(0))
    nc.compile()
    return nc


def get_nc(SQ, S):
    key = (SQ, S, os.environ.get("MHA_FLAGS", ""))
    if key not in _cache:
        _cache[key] = _build_nc(SQ, S)
    return _cache[key]


def kernel(input_x, pad_mask, Wq, bq, Wk, bk, Wv, bv):
    import ml_dtypes
    from concourse.bass_utils import run_bass_kernel_spmd

    BF = ml_dtypes.bfloat16
    input_x = np.asarray(input_x, dtype=np.float32)
    pad_b = np.asarray(pad_mask) != 0
    Wq = np.asarray(Wq, dtype=np.float32)
    Wk = np.asarray(Wk, dtype=np.float32)
    Wv = np.asarray(Wv, dtype=np.float32)
    bq = np.asarray(bq, dtype=np.float32)
    bk = np.asarray(bk, dtype=np.float32)
    bv = np.asarray(bv, dtype=np.float32)

    idxs = [np.flatnonzero(pad_b[b]) for b in range(B)]
    n_max = max(len(i) for i in idxs)
    S = max(128, int(math.ceil(n_max / 128)) * 128)
    SQ = max(1, n_max)
    nc = get_nc(SQ, S)

    NSB = S // 128
    xTs, pads = [], []
    for b in range(B):
        n = len(idxs[b])
        xT = np.zeros((E, SQ), dtype=BF)
        xT[:, :n] = input_x[b][idxs[b]].T.astype(BF)
        xTs.append(xT)
        p = np.zeros((S,), dtype=np.float32)
        p[:n] = 1.0
        pads.append(p.reshape(NSB, 128).T)  # [128, NSB], p-major blocks

    wslices = {}
    for g in range(2):
        sl = slice(g * OC, (g + 1) * OC)
        wslices[g] = (np.ascontiguousarray(Wq[sl].T).astype(BF),
                      np.ascontiguousarray(Wk[sl].T).astype(BF),
                      np.ascontiguousarray(Wv[sl].T).astype(BF),
                      bq[sl].reshape(4, 128).T,
                      bk[sl].reshape(4, 128).T,
                      np.ascontiguousarray(bv[sl]).reshape(1, OC))
    in_maps = []
    for c in range(NCORES):
        b, g = c // 2, c % 2
        wq_t, wk_t, wv_t, bq_s, bk_s, bv_s = wslices[g]
        consts = np.empty((128, NSB + 8), dtype=np.float32)
        consts[:, 0:NSB] = pads[b]
        consts[:, NSB:NSB + 4] = bq_s
        consts[:, NSB + 4:NSB + 8] = bk_s
        in_maps.append({
            "xT": xTs[b], "wqT": wq_t, "wkT": wk_t, "wvT": wv_t,
            "consts": np.ascontiguousarray(consts), "bv": bv_s,
        })

    try:
        res = run_bass_kernel_spmd(nc, in_maps, core_ids=list(range(NCORES)))
    except ModuleNotFoundError:
        # BASS_TRACE set but this axon build lacks the NTFF profile hook —
        # rerun untraced rather than dying.
        os.environ["BASS_NEVER_TRACE"] = "1"
        res = run_bass_kernel_spmd(nc, in_maps, core_ids=list(range(NCORES)))
    if res.exec_time_ns is not None:
        print(f"HW exec time: {res.exec_time_ns} ns")

    out = np.zeros((B, S_FULL, E), dtype=np.float32)
    for c in range(NCORES):
        b, g = c // 2, c % 2
        n = len(idxs[b])
        out[b, idxs[b], g * OC:(g + 1) * OC] = \
            res.results[c]["outT"][:, :n].T.astype(np.float32)
    return out
